# revision 22
# baseline (speedup 1.0000x reference)
"""Trainium2 Bass kernel for nn_AutoEncoderTucker.

Math (reference):
    A   = X @ kron(C_inv, B_inv).T @ G_inv            (encode,  N x R1)
    out = softmax(A) @ relu(G) @ kron(sm(C), sm(B)).T (decode,  N x J*K)

Instead of materializing the (36864 x 2304) Kronecker factors (~700 GFLOP),
the kernel uses the Tucker factorization (~23 GFLOP):
    encode:  Z[(k,r2), n] = sum_j B_inv[r2,j] X[n,(k,j)]     (block-diag matmuls)
             A = Z.T @ M1,  M1[(k,r2),r1] = sum_r3 C_inv[r3,k] G_inv[(r3,r2),r1]
    decode:  W[(k,r2), n] = M2.T @ smA.T, M2[r1,(k,r2)] = sum_r3 smC[k,r3] reluG[r1,(r3,r2)]
             out[n,(k,j)] = W_k.T @ smB.T
M1/M2 are tiny host-precomputed (fold the r3 contraction); r2 is padded
48->64 so two k's pack into one 128-partition tile.

Numerics: the large streamed tensors (X, M1, M2, Z, smA) are fp8-e4m3
with power-of-2 scales; B_inv / smB / the W and output stages stay bf16;
all matmuls accumulate in fp32 PSUM. The scales cancel exactly: the
encode product's scale is divided out inside the softmax exp (scale/bias
args), the decode product's scale is folded into the bf16 smB constant.
Measured absmax-relative error ~5e-3 (threshold 2e-2). fp8 halves HBM
traffic vs bf16 (the kernel is at the HBM roofline): 18.9->9.4 MB for X,
12.6->6.3 MB for M1+M2, per core.

X is pre-transposed on the host into the exact (group, feature-partition,
chunk, sample) tile layout, so every device load is a fully contiguous
~1.6 MB DMA (above the DMA efficiency knee).

Sharding: pure data-parallel over N across the 8 cores (256 rows each);
all small matrices replicated. No collectives.
"""
import numpy as np
import ml_dtypes

BF = ml_dtypes.bfloat16
F8 = ml_dtypes.float8_e4m3

# ---- problem shapes (hardcoded; kernel.py must be self-contained) ----
N, JK = 2048, 36864
J = K = 192
R1, R2, R3 = 256, 48, 48
NCORES = 8
NSH = N // NCORES          # 256 samples per core
R2P = 64                   # r2 padded
T = 96                     # k-pair tiles (2 k's each, 128 padded (k,r2) rows)
NG = 6                     # groups of t
TPG = T // NG              # 16 t per group
FPG = TPG * 2 * J          # 6144 features (and out columns) per group
CKP = K * R2P              # 12288 padded (k,r2) flat size
SX = 32.0                  # X fp8 scale (po2)
SA = 128.0                 # smA fp8 scale (po2)

_CACHE: dict = {}


def _softmax64(t):
    e = np.exp(t - t.max(axis=-1, keepdims=True))
    return e / e.sum(axis=-1, keepdims=True)


def _po2_scale(absmax, target=200.0):
    return float(2.0 ** np.floor(np.log2(target / absmax)))


def host_precompute(B, C, Gm, B_inv, C_inv, G_inv, m_fp8=True):
    f64 = np.float64
    B, C, Gm = np.asarray(B, f64), np.asarray(C, f64), np.asarray(Gm, f64)
    B_inv = np.asarray(B_inv, f64)
    C_inv = np.asarray(C_inv, f64)
    G_inv = np.asarray(G_inv, f64)

    smB, smC = _softmax64(B), _softmax64(C)
    reluG = np.maximum(Gm, 0.0)

    # M1[k*64+r2, r1] = sum_r3 C_inv[r3,k] * G_inv[r3*48+r2, r1]; fp8 x SM1
    G3 = G_inv.reshape(R3, R2, R1)
    M1c = np.einsum('rk,rsp->ksp', C_inv, G3)
    sm1 = _po2_scale(np.abs(M1c).max())
    M1p = np.zeros((K, R2P, R1), f64)
    M1p[:, :R2, :] = M1c * sm1
    # pre-rearranged to [p, t, r1] so the device load is a fully
    # contiguous per-partition DMA (the (i p) r gather had 256B segments
    # at fp8 -- below SDMA line-rate minimum)
    M1p = M1p.reshape(T, 128, R1).transpose(1, 0, 2)
    M1p = np.ascontiguousarray(M1p.astype(F8 if m_fp8 else BF))

    # M2[r1, k*64+r2] = sum_r3 smC[k,r3] * reluG[r1, r3*48+r2]; fp8 x SM2
    G3d = reluG.reshape(R1, R3, R2)
    M2c = np.einsum('kr,prs->pks', smC, G3d)
    sm2 = _po2_scale(np.abs(M2c).max())
    M2p = np.zeros((R1, K, R2P), f64)
    M2p[:, :, :R2] = M2c * sm2
    M2p = M2p.reshape(R1, CKP)
    # DoubleRow packing: [p, h, ckr2] with h = r1-half (contraction pairs)
    M2p = M2p.reshape(2, 128, CKP).transpose(1, 0, 2)
    M2p = np.ascontiguousarray(M2p.astype(F8 if m_fp8 else BF))

    # S1 weights: B_inv.T (j, r2) zero-padded to 64 cols, as block-diagonal
    # lhsT for the 3-tile / 2-k X layout.  w1 = [WA | WB | WC] (128, 384);
    # out cols 0:64 = k-even, 64:128 = k-odd, every matmul dst at partition 0.
    BinvTp = np.zeros((J, R2P), f64)
    BinvTp[:, :R2] = B_inv.T
    w1 = np.zeros((128, 384), f64)
    w1[0:128, 0:64] = BinvTp[0:128]                 # WA: k-even, j 0:128
    w1[0:64, 128 + 0:128 + 64] = BinvTp[128:192]    # WB hi: k-even, j 128:192
    w1[64:128, 128 + 64:128 + 128] = BinvTp[0:64]   # WB lo: k-odd, j 0:64
    w1[0:128, 256 + 64:256 + 128] = BinvTp[64:192]  # WC: k-odd, j 64:192
    w1 = np.ascontiguousarray(w1.astype(BF))

    # smB.T / (SA*SM2), duplicated at partition bases 0 and 64
    smbt2 = np.zeros((128, J), f64)
    smbt2[0:R2] = smB.T / (SA * sm2)
    smbt2[64:64 + R2] = smB.T / (SA * sm2)
    smbt2 = np.ascontiguousarray(smbt2.astype(BF))

    ident = np.eye(128, dtype=np.float32).astype(BF)
    # softmax input scale: A_psum = SX*sm1*A -> exp scale 1/(SX*sm1)
    escale = np.full((128, 1), 1.0 / (SX * sm1), np.float32)
    return {"w1": w1, "m1p": M1p, "m2p": M2p, "smbt2": smbt2, "ident": ident,
            "escale": escale}


def build_nc(n_groups=NG, reps=1, x_fp8=True, m_fp8=True, out_sync=True, m2_early=True, ph1=True, ph2=True, deep=False, swp=True, p2_wtponly=False, p2_nocopy=False, fuse=False, gcopy=False, dr=False):
    """Build + bacc-compile the per-core Tile kernel.

    reps>1 unrolls the identical body multiple times in one NEFF; used by
    the timing harness to extract per-rep HW time as a wall-clock slope
    (per-call dispatch overhead through axon/PJRT is ~75ms and would
    otherwise swamp the sub-ms kernel).
    """
    import concourse.bass as bass
    import concourse.bacc as bacc
    import concourse.mybir as mybir
    from concourse import tile

    f32 = mybir.dt.float32
    bf16 = mybir.dt.bfloat16
    fp8 = mybir.dt.float8e4
    PS = bass.MemorySpace.PSUM
    AX = mybir.AxisListType.X
    AF = mybir.ActivationFunctionType
    nt = n_groups * TPG

    nc = bacc.Bacc(None, target_bir_lowering=False, debug=False,
                   num_devices=NCORES)

    xdt = fp8 if x_fp8 else bf16
    mdt = fp8 if m_fp8 else bf16
    x = nc.dram_tensor("x", [NG, 128, 3 * TPG, 256], xdt, kind="ExternalInput")
    m1p = nc.dram_tensor("m1p", [128, T, R1], mdt, kind="ExternalInput")
    m2p = nc.dram_tensor("m2p", [128, 2, CKP], mdt, kind="ExternalInput")
    w1 = nc.dram_tensor("w1", [128, 384], bf16, kind="ExternalInput")
    smbt2 = nc.dram_tensor("smbt2", [128, J], bf16, kind="ExternalInput")
    ident = nc.dram_tensor("ident", [128, 128], bf16, kind="ExternalInput")
    escale = nc.dram_tensor("escale", [128, 1], f32, kind="ExternalInput")
    out = nc.dram_tensor("out", [NSH, JK], bf16, kind="ExternalOutput")

    with tile.TileContext(nc) as tc:
      for rep in range(reps):
        with tc.tile_pool(name=f"const{rep}", bufs=1) as cpool:
            w1_t = cpool.tile([128, 384], bf16, tag="w1", name="w1")
            nc.gpsimd.dma_start(w1_t[:], w1[:])
            smbt2_t = cpool.tile([128, J], bf16, tag="smbt2", name="smbt2")
            nc.gpsimd.dma_start(smbt2_t[:], smbt2[:])
            ident_t = cpool.tile([128, 128], bf16, tag="ident", name="ident")
            nc.gpsimd.dma_start(ident_t[:], ident[:])
            escale_t = cpool.tile([128, 1], f32, tag="escale", name="escale")
            nc.gpsimd.dma_start(escale_t[:], escale[:])
            # smA.T halves stacked [r1-in-half, h, n], persistent
            smat2 = cpool.tile([128, 2, 256], mdt, tag="smat2", name="smat2")
            smat = [smat2[:, h, :] for h in range(2)]
            # M1 / M2 fully resident (3.1 MB each at fp8). Both stream in
            # during phase 1 (its DMA load is light at fp8); chunked so
            # early tiles unblock compute immediately.
            m1f = cpool.tile([128, T, R1], mdt, tag="m1f", name="m1f")
            m2ft = cpool.tile([128, 2, CKP], mdt, tag="m2ft", name="m2ft")
            m2f = [m2ft[:, h, :] for h in range(2)]
            if not ph1:
                for h in range(2):
                    nc.vector.memset(smat[h][:], 0)

            # ---------------- phase 1: encode ----------------
            with (
                tc.tile_pool(name=f"xt{rep}", bufs=2) as xt_pool,
                tc.tile_pool(name=f"ztsb{rep}", bufs=3) as zt_pool,
                tc.tile_pool(name=f"smx{rep}", bufs=1) as smx_pool,
                tc.tile_pool(name=f"tp_ps{rep}", bufs=2, space=PS) as tp_ps,
                tc.tile_pool(name=f"zt_ps{rep}", bufs=2, space=PS) as zt_ps,
                tc.tile_pool(name=f"a_ps{rep}", bufs=1, space=PS) as a_ps,
            ):
                a_psum = [a_ps.tile([128, R1], f32, tag=f"a{nb}", name=f"a{nb}")
                          for nb in range(2)]
                for g in range(n_groups):
                    # host pre-transposed: one contiguous 1.6 MB load per group
                    xt3 = xt_pool.tile([128, 3 * TPG, 256], xdt, tag="xt",
                                       name="xt")
                    nc.sync.dma_start(xt3[:], x[g])
                    if g == 0:
                        for q in range(4):
                            tq = max(nt // 4, 1)
                            lo, hi = q * tq, min((q + 1) * tq, nt)
                            if lo < hi:
                                nc.gpsimd.dma_start(
                                    m1f[:, lo:hi, :], m1p[:, lo:hi, :])
                        if m2_early:
                            for h in range(2):
                                for q in range(2):
                                    lo, hi = q * (CKP // 2), (q + 1) * (CKP // 2)
                                    nc.gpsimd.dma_start(
                                        m2ft[:, h, lo:hi],
                                        m2p[:, h, lo:hi])
                    for i in range(TPG if ph1 else 0):
                        t = g * TPG + i
                        xt = [xt3[:, i * 3 + c, :] for c in range(3)]
                        ztp = zt_ps.tile([128, 256], f32, tag="ztp", name="ztp")
                        nc.tensor.matmul(ztp[:], w1_t[:, 0:128],
                                         xt[0], start=True, stop=False)
                        nc.tensor.matmul(ztp[:], w1_t[:, 128:256],
                                         xt[1], start=False, stop=False)
                        nc.tensor.matmul(ztp[:], w1_t[:, 256:384],
                                         xt[2], start=False, stop=True)
                        zts = zt_pool.tile([128, 256], mdt, tag="zt", name="zt")
                        nc.vector.tensor_copy(zts[:], ztp[:])
                        for nb in range(2):
                            nc.tensor.matmul(
                                a_psum[nb][:],
                                zts[:, nb * 128:(nb + 1) * 128],
                                m1f[:, t, :],
                                start=(t == 0), stop=(t == nt - 1))
                # softmax along r1 (free dim) + PE transpose into smat
                # a_psum holds SX*SM1*A; exp((a - max)*escale) removes the
                # scale exactly; smA is stored x128 (folded into smbt2).
                for nb in range(2 if ph1 else 0):
                    nmax = smx_pool.tile([128, 1], f32, tag=f"nmax{nb}", name=f"nmax{nb}")
                    nc.vector.reduce_max(nmax[:], a_psum[nb][:], axis=AX, negate=True)
                    nmax2 = smx_pool.tile([128, 1], f32, tag=f"nmax2{nb}", name=f"nmax2{nb}")
                    nc.vector.tensor_scalar_mul(nmax2[:], nmax[:], escale_t[:])
                    esum = smx_pool.tile([128, 1], f32, tag=f"esum{nb}", name=f"esum{nb}")
                    expt = smx_pool.tile([128, 256], f32, tag=f"expt{nb}", name=f"expt{nb}")
                    nc.scalar.activation(expt[:], a_psum[nb][:], AF.Exp,
                                         bias=nmax2[:], scale=escale_t[:],
                                         accum_out=esum[:])
                    rinv = smx_pool.tile([128, 1], f32, tag=f"rinv{nb}", name=f"rinv{nb}")
                    nc.vector.reciprocal(rinv[:], esum[:])
                    rinv2 = smx_pool.tile([128, 1], f32, tag=f"rinv2{nb}", name=f"rinv2{nb}")
                    nc.vector.tensor_scalar_mul(rinv2[:], rinv[:], SA)
                    sma = smx_pool.tile([128, 256], bf16, tag=f"sma{nb}", name=f"sma{nb}")
                    nc.vector.tensor_scalar_mul(sma[:], expt[:], rinv2[:])
                    for h in range(2):
                        tp = tp_ps.tile([128, 128], bf16, tag="tp", name="tp")
                        nc.tensor.transpose(
                            tp[:], sma[:, h * 128:(h + 1) * 128], ident_t[:])
                        nc.vector.tensor_copy(
                            smat[h][:, nb * 128:(nb + 1) * 128], tp[:])

            # ---------------- phase 2: decode ----------------
            with (
                tc.tile_pool(name=f"wtsb{rep}", bufs=(6 if deep else 3)) as wt_pool,
                tc.tile_pool(name=f"osb{rep}", bufs=2) as osb_pool,
                tc.tile_pool(name=f"wt_ps{rep}", bufs=(4 if deep else 2), space=PS) as wt_ps,
                tc.tile_pool(name=f"o_ps{rep}", bufs=4, space=PS) as o_ps,
            ):
                if not m2_early:
                    for h in range(2):
                        for q in range(2):
                            lo, hi = q * (CKP // 2), (q + 1) * (CKP // 2)
                            nc.gpsimd.dma_start(
                                m2ft[:, h, lo:hi],
                                m2p[:, h, lo:hi])
                def emit_ops(pend):
                    if p2_wtponly:
                        return
                    # out-stage for a prior tile: tensor consumes wts copies
                    # that vector finished while tensor ran the next wtp pair
                    wts, osb_p, g_p, i_p = pend
                    for nb in range(2):
                        op = o_ps.tile([128, 2 * J], f32, tag="op", name="op")
                        for kk in range(2):
                            b = kk * 64 if fuse else 0
                            lhs = (wts[kk][:, nb * 128:(nb + 1) * 128] if fuse
                                   else wts[kk][0:64, nb * 128:(nb + 1) * 128])
                            nc.tensor.matmul(
                                op[:, kk * J:(kk + 1) * J], lhs,
                                smbt2_t[b:b + 64, :],
                                start=(kk == 0), stop=(kk == 1))
                        if p2_nocopy:
                            continue
                        if nb == 0:
                            nc.vector.tensor_copy(
                                osb_p[nb][:, i_p * 2 * J:(i_p + 1) * 2 * J],
                                op[:])
                        else:
                            nc.scalar.copy(
                                osb_p[nb][:, i_p * 2 * J:(i_p + 1) * 2 * J], op[:])
                    if (not p2_nocopy) and i_p == TPG - 1:
                        for nb in range(2):
                            (nc.sync if out_sync else nc.gpsimd).dma_start(
                                out[nb * 128:(nb + 1) * 128,
                                    g_p * FPG:(g_p + 1) * FPG],
                                osb_p[nb][:])

                pend = None
                for g in range(n_groups if ph2 else 0):
                    osb = (None if (p2_wtponly or p2_nocopy) else
                           [osb_pool.tile([128, FPG], bf16, tag=f"osb{nb}",
                                          name=f"osb{nb}") for nb in range(2)])
                    for i in range(TPG):
                        t = g * TPG + i
                        # one (128,256) matmul pair; then partition-shifted
                        # copies put each k-half at base 0
                        wtp = wt_ps.tile([128, 256], f32, tag="wtp", name="wtp")
                        if dr:
                            nc.tensor.matmul(
                                wtp[:], m2ft[:, :, t * 128:(t + 1) * 128],
                                smat2[:], start=True, stop=True,
                                perf_mode=mybir.MatmulPerfMode.DoubleRow)
                        else:
                            for h in range(2):
                                nc.tensor.matmul(wtp[:],
                                                 m2f[h][:, t * 128:(t + 1) * 128],
                                                 smat[h][:],
                                                 start=(h == 0), stop=(h == 1))
                        if fuse:
                            wtf = wt_pool.tile([128, 256], bf16, tag="wtf",
                                               name="wtf")
                            nc.vector.tensor_copy(wtf[:], wtp[:])
                            wts = [wtf[0:64, :], wtf[64:128, :]]
                        else:
                            wts = [wt_pool.tile([64, 256], bf16, tag=f"wt{kk}",
                                                name=f"wt{kk}") for kk in range(2)]
                            nc.vector.tensor_copy(wts[0][:], wtp[0:64, :])
                            if gcopy:
                                nc.scalar.copy(wts[1][:], wtp[64:128, :])
                            else:
                                nc.vector.tensor_copy(wts[1][:], wtp[64:128, :])
                        if swp:
                            if pend is not None:
                                emit_ops(pend)
                            pend = (wts, osb, g, i)
                        else:
                            emit_ops((wts, osb, g, i))
                if pend is not None:
                    emit_ops(pend)
    nc.compile()
    return nc


def _get_nc(n_groups=NG):
    key = ("nc", n_groups)
    if key not in _CACHE:
        _CACHE[key] = build_nc(n_groups)
    return _CACHE[key]


def make_in_maps(X, consts, x_fp8=True):
    X = np.asarray(X, np.float32)
    maps = []
    for c in range(NCORES):
        Xs = X[c * NSH:(c + 1) * NSH]
        # [g, f, cc, n] <- Xs[n, g*6144 + cc*128 + f]  (x SX, fp8)
        xp = (Xs.reshape(NSH, NG, 3 * TPG, 128) * SX).transpose(1, 3, 2, 0)
        xp = np.ascontiguousarray(xp.astype(F8 if x_fp8 else BF))
        maps.append({"x": xp, **consts})
    return maps


def run(inputs, trace=False, n_groups=NG, **kwargs):
    """Run on 8 cores; returns (full fp32 output, BassKernelResults)."""
    from concourse.bass_utils import run_bass_kernel_spmd
    consts = host_precompute(inputs["B"], inputs["C"], inputs["G"],
                             inputs["B_inv"], inputs["C_inv"], inputs["G_inv"])
    in_maps = make_in_maps(inputs["X"], consts)
    nc = _get_nc(n_groups)
    res = run_bass_kernel_spmd(nc, in_maps, core_ids=list(range(NCORES)),
                               trace=trace, **kwargs)
    outs = [res.results[c]["out"] for c in range(NCORES)]
    full = np.concatenate(outs, axis=0).astype(np.float32)
    return full, res


def kernel(X, B, C, G, B_inv, C_inv, G_inv):
    full, _ = run(dict(X=X, B=B, C=C, G=G,
                       B_inv=B_inv, C_inv=C_inv, G_inv=G_inv))
    return full
